# revision 18
# baseline (speedup 1.0000x reference)
"""Trainium2 Bass kernel for the DiscreteAutoregressiveFlow sampling problem.

Reference semantics (B=1024, L=1024, V=20, D=128):
    scan over t:  net = state @ W_out + b          [B, 2V]
                  m = argmax(net[:, :V]); s = argmax(net[:, V:])
                  u = ((a_t - m) * inv(s)) % V     (a_t = index of one-hot x_t,
                                                    inv(s)=mult. inverse mod V, 0 if non-coprime)
                  out_t = one_hot(u); state += emb[u]
Outputs ys[b, t] = one_hot(u_t).

Sharding: pure data-parallel over batch: 8 cores x 128 rows. Each core runs the
full L-step scan for its 128 rows (batch rows on SBUF partitions).

Key numerical property (validated off-line): keeping the full [B, D] state and
re-doing state @ W_out each step tracks the fp32 reference trajectory far
better than mathematically equivalent reformulations, because state += emb[u]
is bitwise-identical to the reference and only per-step matmul rounding
differs.

Per-step data flow on a core (batch rows b on partitions):
  stateT_sbuf[d, b] <- copy state_psum          (DVE)
  net_psum[b, j] = stateT.T @ W_out             (PE, start=True)
  net_psum[b, j] += ones.T @ b_row              (PE, adds bias, stop=True)
  net_sbuf <- copy net_psum                     (DVE)
  mx[b, 2] = segmented max over net [b, 2, 20]  (DVE)
  df = net - broadcast(mx)                      (DVE)  == 0 exactly at argmax
  msk = (df >= 0) * CMASK                       (DVE)  CMASK head0: 20-j
                                                       head1: 32*(20-j) + p[j]
  r[b, 2] = segmented max over msk              (DVE)  r0 = 20-m
                                                       r1 = 32*(20-s)+p[s]
  w = r1 mod 32            = p[s] = (20-inv[s])%20     (DVE)
  tp = (A5_t - r0) * w     = (m-a)*(20-inv[s])         (DVE)  A5 = 20-a
  u = (tp + 380) mod 20    = ((a-m)*inv[s]) % 20       (DVE)
  oh = (IOTA32 == u)  -> written into ys ring          (DVE)
  ohT_psum = oh.T                                      (PE transpose)
  ohT_sbuf <- copy ohT_psum                            (DVE)
  state_psum[d, b] += emb32.T @ ohT  (= emb[u] row)    (PE, accumulating)

A5 (= 20 - a_t) is produced on-device from the one-hot x input:
  A5[:, t] = sum_v x[b, t, v] * (20 - v)   via chunked mult + segmented reduce.
"""

import numpy as np

B, L, V, D = 1024, 1024, 20, 128
NCORES = 8
BLOC = B // NCORES  # 128 batch rows per core
J2 = 2 * V          # 40

_CACHE = {}

# column offsets of each constant inside the single consts blob [128, CONSTS_W]
_COFF = {'emb32': 0, 'wout': 128, 'brow128': 168, 'cmask': 208,
         'iota32': 248, 'c20j': 280, 'ident': 300, 'ones1': 428}
CONSTS_W = 556


def _build_consts_blob(emb, W_out, b):
    """Host-side constants packed into one [128, CONSTS_W] fp32 blob."""
    f32 = np.float32
    blob = np.zeros((128, CONSTS_W), dtype=f32)
    o = _COFF
    blob[:V, o['emb32']:o['emb32'] + D] = emb
    blob[:, o['wout']:o['wout'] + J2] = W_out
    blob[:, o['brow128']:o['brow128'] + J2] = b[None, :]
    # inverse-mod-20 table; p[s] = (20 - inv[s]) % 20 so that
    # u = ((m-a) * p[s]) % 20 == ((a-m)*inv[s]) % 20, and p=0 for non-coprime s
    inv = np.zeros(V, dtype=np.int64)
    for k in range(1, V):
        if np.gcd(k, V) == 1:
            inv[k] = pow(k, -1, V)
    p = (V - inv) % V
    j = np.arange(V)
    c0 = (V - j).astype(f32)              # 20 - j  (strictly decreasing)
    c1 = (32 * (V - j) + p).astype(f32)   # strictly decreasing, r1 mod 32 = p
    blob[:, o['cmask']:o['cmask'] + J2] = np.concatenate([c0, c1])[None, :]
    # cols >= 20 get a sentinel so the fused (j + 20q == v) compare can't
    # alias to j = u + 20
    iota32 = np.arange(32, dtype=f32)
    iota32[V:] = 1000.0
    blob[:, o['iota32']:o['iota32'] + 32] = iota32[None, :]
    blob[:, o['c20j']:o['c20j'] + V] = (V - np.arange(V, dtype=f32))[None, :]
    blob[:, o['ident']:o['ident'] + 128] = np.eye(128, dtype=f32)
    blob[0, o['ones1']:o['ones1'] + 128] = 1.0
    return blob


def _build_module(nsteps):
    import concourse.bass as bass
    import concourse.bacc as bacc
    import concourse.mybir as mybir
    import concourse.tile as tile

    f32 = mybir.dt.float32
    i32 = mybir.dt.int32
    nc = bacc.Bacc()

    x_d = nc.declare_dram_parameter("xloc", [BLOC, nsteps, V], f32, isOutput=False)
    consts_d = nc.declare_dram_parameter("consts", [128, CONSTS_W], f32,
                                         isOutput=False)
    ys_d = nc.declare_dram_parameter("ys", [BLOC, nsteps, V], f32, isOutput=True)

    XCH = min(128, nsteps)          # x prefetch / A5 chunk, in steps
    nxch = (nsteps + XCH - 1) // XCH
    YCH = min(64, nsteps)           # ys flush chunk, in steps
    RING = 2 * YCH

    sub = mybir.AluOpType.subtract
    mult = mybir.AluOpType.mult
    add = mybir.AluOpType.add
    band = mybir.AluOpType.bitwise_and
    shr = mybir.AluOpType.arith_shift_right
    is_ge = mybir.AluOpType.is_ge
    is_eq = mybir.AluOpType.is_equal
    axX = mybir.AxisListType.X

    with tile.TileContext(nc) as tc:
        with (
            tc.tile_pool(name="persist", bufs=1) as pp,
            tc.tile_pool(name="xstage", bufs=2) as xp,
            tc.tile_pool(name="psum", bufs=1, space="PSUM") as pspool,
        ):
            # constants: one blob, one DMA, one semaphore (ISA structs have
            # few wait slots; scattered const DMAs overflow them)
            cblob = pp.tile([128, CONSTS_W], f32, tag="cblob")
            nc.sync.dma_start(out=cblob[:], in_=consts_d[:])
            o = _COFF
            emb32 = cblob[0:32, o['emb32']:o['emb32'] + D]
            wout = cblob[:, o['wout']:o['wout'] + J2]
            brow128 = cblob[:, o['brow128']:o['brow128'] + J2]
            brow = cblob[0:1, o['brow128']:o['brow128'] + J2]
            cmask = cblob[:, o['cmask']:o['cmask'] + J2]
            iota32 = cblob[:, o['iota32']:o['iota32'] + 32]
            c20j = cblob[:, o['c20j']:o['c20j'] + V]
            ident = cblob[:, o['ident']:o['ident'] + 128]
            ones1 = cblob[0:1, o['ones1']:o['ones1'] + 128]
            # all-engine barrier so the const-DMA wait lands here once
            tc.strict_bb_all_engine_barrier()

            # persistent working tiles
            a5 = pp.tile([BLOC, nsteps], f32, tag="a5")
            ysring = pp.tile([BLOC, RING, 32], f32, tag="ysring")
            stateT = pp.tile([D, BLOC], f32, tag="stateT")
            netb = pp.tile([BLOC, J2], f32, tag="netb")
            dfm = pp.tile([BLOC, J2], f32, tag="dfm")
            msk = pp.tile([BLOC, J2], f32, tag="msk")
            mx = pp.tile([BLOC, 2], f32, tag="mx")
            rr = pp.tile([BLOC, 2], f32, tag="rr")
            zf = pp.tile([BLOC, 1], f32, tag="zf")
            kf = pp.tile([BLOC, 1], f32, tag="kf")
            pf = pp.tile([BLOC, 1], f32, tag="pf")
            tpf = pp.tile([BLOC, 1], f32, tag="tpf")
            vf = pp.tile([BLOC, 1], f32, tag="vf")
            yf = pp.tile([BLOC, 1], f32, tag="yf")
            qf = pp.tile([BLOC, 1], f32, tag="qf")
            q20f = pp.tile([BLOC, 1], f32, tag="q20f")
            ohT = pp.tile([32, BLOC], f32, tag="ohT")

            net_ps = pspool.tile([BLOC, J2], f32, tag="net_ps")
            ohT_ps = pspool.tile([32, BLOC], f32, tag="ohT_ps")
            state_ps = pspool.tile([D, BLOC], f32, tag="state_ps")

            # ---- A5 pre-pass: A5[:, t] = sum_v x[:, t, v] * (20 - v) ----
            for c in range(nxch):
                t0 = c * XCH
                ncols = min(XCH, nsteps - t0)
                xt = xp.tile([BLOC, XCH, V], f32, tag="xt")
                xm = xp.tile([BLOC, XCH, V], f32, tag="xm")
                nc.sync.dma_start(out=xt[:, :ncols, :], in_=x_d[:, t0:t0 + ncols, :])
                nc.vector.tensor_tensor(
                    out=xm[:, :ncols, :], in0=xt[:, :ncols, :],
                    in1=c20j[:].unsqueeze(1).broadcast_to((BLOC, ncols, V)),
                    op=mult)
                nc.vector.reduce_sum(out=a5[:, t0:t0 + ncols], in_=xm[:, :ncols, :],
                                     axis=axX)

            # ---- the scan ----
            for t in range(nsteps):
                if t == 0:
                    nc.vector.tensor_copy(netb[:], brow128[:])
                else:
                    nc.vector.tensor_copy(stateT[:], state_ps[:])
                    nc.tensor.matmul(net_ps[:], stateT[:], wout[:],
                                     start=True, stop=False)
                    nc.tensor.matmul(net_ps[:], ones1[:], brow[:],
                                     start=False, stop=True)
                    nc.vector.tensor_copy(netb[:], net_ps[:])

                # segmented argmax encode over both heads
                nc.vector.reduce_max(out=mx[:], in_=netb[:].rearrange(
                    "b (h v) -> b h v", h=2), axis=axX)
                nc.vector.tensor_tensor(
                    out=dfm[:].rearrange("b (h v) -> b h v", h=2),
                    in0=netb[:].rearrange("b (h v) -> b h v", h=2),
                    in1=mx[:].unsqueeze(2).broadcast_to((BLOC, 2, V)),
                    op=sub)
                nc.vector.scalar_tensor_tensor(
                    out=msk[:], in0=dfm[:], scalar=0.0, in1=cmask[:],
                    op0=is_ge, op1=mult)
                nc.vector.reduce_max(out=rr[:], in_=msk[:].rearrange(
                    "b (h v) -> b h v", h=2), axis=axX)
                # mod/divide are not in the DVE ISA. Pure-fp32 floor via the
                # round-to-int trick  RN(y) = (y + 2^23) - 2^23, with offsets
                # chosen so y is always strictly within (-0.5, 0.5) of the
                # target integer (exact & tie-free in IEEE RN):
                #   k = floor(r1/32) = 20-s;  p = r1 - 32k  (= p[s])
                #   v = (A5-r0)*p + 380 = (m-a)*p + 380, in [19, 741]
                #   q = floor(v*3277/65536) = v//20
                #   oh[j] = (j + 20q == v)  <=>  j == u = v mod 20
                nc.vector.tensor_scalar(
                    out=zf[:], in0=rr[:, 1:2], scalar1=0.03125, op0=mult,
                    scalar2=-0.484375, op1=add)
                nc.vector.tensor_scalar(
                    out=kf[:], in0=zf[:], scalar1=8388608.0, op0=add,
                    scalar2=8388608.0, op1=sub)
                nc.vector.tensor_scalar(
                    out=pf[:], in0=kf[:], scalar1=-32.0, op0=mult,
                    scalar2=rr[:, 1:2], op1=add)
                nc.vector.tensor_scalar(
                    out=tpf[:], in0=a5[:, t:t + 1], scalar1=rr[:, 0:1],
                    op0=sub, scalar2=pf[:], op1=mult)
                nc.vector.tensor_scalar(
                    out=vf[:], in0=tpf[:], scalar1=380.0, scalar2=None,
                    op0=add)
                nc.vector.tensor_scalar(
                    out=yf[:], in0=vf[:], scalar1=0.0500030517578125,
                    op0=mult, scalar2=-0.49999237060546875, op1=add)
                nc.vector.tensor_scalar(
                    out=qf[:], in0=yf[:], scalar1=8388608.0, op0=add,
                    scalar2=8388608.0, op1=sub)
                nc.vector.tensor_scalar(
                    out=q20f[:], in0=qf[:], scalar1=20.0, scalar2=None,
                    op0=mult)
                ohs = ysring[:, t % RING, :]
                nc.vector.tensor_scalar(
                    out=ohs, in0=iota32[:], scalar1=q20f[:], op0=add,
                    scalar2=vf[:], op1=is_eq)

                # state += emb[u]  via transpose + one-hot matmul
                nc.tensor.transpose(ohT_ps[:], ohs, ident[:])
                nc.vector.tensor_copy(ohT[:], ohT_ps[:])
                nc.tensor.matmul(state_ps[:], emb32[:], ohT[:],
                                 start=(t == 0), stop=(t == nsteps - 1),
                                 skip_group_check=True)

                # flush ys every YCH steps
                if (t + 1) % YCH == 0:
                    h0 = (t + 1 - YCH) % RING
                    nc.sync.dma_start(
                        out=ys_d[:, t + 1 - YCH:t + 1, :],
                        in_=ysring[:, h0:h0 + YCH, :V])
            if nsteps % YCH:
                tdone = (nsteps // YCH) * YCH
                h0 = tdone % RING
                nc.sync.dma_start(
                    out=ys_d[:, tdone:nsteps, :],
                    in_=ysring[:, h0:h0 + (nsteps - tdone), :V])

    nc.finalize()
    return nc


def _get_module(nsteps):
    if nsteps not in _CACHE:
        _CACHE[nsteps] = _build_module(nsteps)
    return _CACHE[nsteps]


def _make_in_maps(x, emb, W_out, b, nsteps):
    f32 = np.float32
    blob = _build_consts_blob(np.asarray(emb, f32), np.asarray(W_out, f32),
                              np.asarray(b, f32))
    in_maps = []
    for c in range(NCORES):
        xl = np.ascontiguousarray(x[c * BLOC:(c + 1) * BLOC, :nsteps, :], f32)
        in_maps.append(dict(xloc=xl, consts=blob))
    return in_maps


def kernel(x, emb, W_out, b):
    from concourse.bass_utils import run_bass_kernel_spmd

    x = np.asarray(x, np.float32)
    emb = np.asarray(emb, np.float32)
    W_out = np.asarray(W_out, np.float32)
    b = np.asarray(b, np.float32)

    nsteps = x.shape[1]
    nc = _get_module(nsteps)
    in_maps = _make_in_maps(x, emb, W_out, b, nsteps)
    res = run_bass_kernel_spmd(nc, in_maps, list(range(NCORES)))
    out = np.concatenate([res.results[c]["ys"] for c in range(NCORES)], axis=0)
    return out.astype(np.float32)


# revision 38
# speedup vs baseline: 2089.6400x; 2089.6400x over previous
"""Trainium2 Bass kernel for the DiscreteAutoregressiveFlow sampling problem.

Reference semantics (B=1024, L=1024, V=20, D=128):
    scan over t:  net = state @ W_out + b          [B, 2V]
                  m = argmax(net[:, :V]); s = argmax(net[:, V:])
                  u = ((a_t - m) * inv(s)) % V     (a_t = index of one-hot x_t,
                                                    inv(s) = mult. inverse mod V,
                                                    0 if s not coprime with V)
                  out_t = one_hot(u); state += emb[u]
Outputs ys[b, t] = one_hot(u_t).

Sharding: pure data-parallel over batch: 8 cores x 128 rows; batch rows live on
SBUF partitions; each core runs the full L-step scan for its 128 rows.

Numerics (validated off-line): the [B, D] state is accumulated EXACTLY as the
reference does (state += emb[u] is bitwise-identical; emb-row select happens
through an exact one-hot matmul), so only per-step matmul rounding differs
from the fp32 reference trajectory (~1e-4 of argmax decisions flip).

Structure per step t (b = batch on partitions):
  DVE:  mx   = segmented max over net_ps[A] [b, 2, 20]
        dfm  = net - bcast(mx)               (== 0 exactly at the argmax)
        msk[:, 0:40]  = (dfm >= 0) * [C0|C0]  C0[j] = 20-j (decreasing ->
        msk[:, 40:60] = (dfm[20:] >= 0)*C2       first-index tie-break)
                                              C2[j] = (20-j) + p[j]/64
        rr3  = segmented max over msk [b,3,20]   -> 20-m, 20-s, (20-s)+p[s]/64
        pf   = (rr3[2]-rr3[1])*64  = p[s] = (20 - inv[s]) % 20
        tpf  = (A5 - rr3[0])*pf    = (m-a)*p      in [-361, 361]
        yf   = tpf*(3277/65536) + d'   ; exact fp32, strictly inside
        qf   = (yf + 2^23) - 2^23      = floor((tpf+380)/20)   (RN trick)
        q20f = qf*20
        oh[j]= ((j-380) + q20f == tpf)  ->  one-hot of u, into the ys ring
        vt   = 32x32 block transpose of oh     (DVE stream transpose)
  ACT:  stateT <- copy state_ps                 (off critical path)
  PE:   mm4 x4: net_ps[B][32q:32q+32] = vt-chunk^T @ EWb (start=True)
        mm1:    net_ps[B] += stateT^T @ W_out   (state_t @ W, next step's net)
        mm3 x4: state_ps[:, 32q:32q+32] += emb-chunk^T @ vt-chunk (exact row add)
  where EWb[v] = emb[v] @ W_out + b (host fp64->fp32), so net(t+1) =
  state_t @ W + b + EW[u_t] = state_{t+1} @ W + b up to one extra rounding.

A5 (= 20 - a_t) is produced on-device from the one-hot x input by a chunked
multiply + segmented-reduce pre-pass; ys one-hots are flushed from a double
ring every 64 steps.
"""

import numpy as np

B, L, V, D = 1024, 1024, 20, 128
NCORES = 8
BLOC = B // NCORES  # 128 batch rows per core
J2 = 2 * V          # 40

_CACHE = {}

# column offsets inside the single consts blob [128, CONSTS_W]
_COFF = {'emb32': 0, 'wout': 128, 'brow128': 168, 'cmab': 208, 'c2': 248,
         'iota32': 268, 'c20j': 300, 'ewb32': 320, 'ident': 360}
CONSTS_W = 488


def _build_consts_blob(emb, W_out, b):
    """Host-side constants packed into one [128, CONSTS_W] fp32 blob."""
    f32 = np.float32
    blob = np.zeros((128, CONSTS_W), dtype=f32)
    o = _COFF
    blob[:V, o['emb32']:o['emb32'] + D] = emb
    blob[:, o['wout']:o['wout'] + J2] = W_out
    blob[:, o['brow128']:o['brow128'] + J2] = b[None, :]
    # EWb = emb @ W_out + b in fp64, rounded once
    ewb = (emb.astype(np.float64) @ W_out.astype(np.float64)
           + b.astype(np.float64)).astype(f32)
    blob[:V, o['ewb32']:o['ewb32'] + J2] = ewb
    blob[:, o['ident']:o['ident'] + 128] = np.eye(128, dtype=f32)
    # p[s] = (20 - inv[s]) % 20 so that u = ((m-a)*p[s]) % 20, p=0 for
    # non-coprime s (matching the reference's INV_P zero rows)
    inv = np.zeros(V, dtype=np.int64)
    for k in range(1, V):
        if np.gcd(k, V) == 1:
            inv[k] = pow(k, -1, V)
    p = (V - inv) % V
    j = np.arange(V)
    c0 = (V - j).astype(f32)
    blob[:, o['cmab']:o['cmab'] + J2] = np.concatenate([c0, c0])[None, :]
    blob[:, o['c2']:o['c2'] + V] = (c0 + p.astype(f32) / 64.0)[None, :]
    # iota32 holds j - 380 for the fused (j-380 + 20q == tp) compare; cols
    # >= 20 get a sentinel that can never match (tp <= 361 < 620)
    iota32 = np.arange(32, dtype=f32) - 380.0
    iota32[V:] = 1000.0
    blob[:, o['iota32']:o['iota32'] + 32] = iota32[None, :]
    blob[:, o['c20j']:o['c20j'] + V] = (V - np.arange(V, dtype=f32))[None, :]
    return blob


def _build_module(nsteps):
    import concourse.bacc as bacc
    import concourse.mybir as mybir
    import concourse.tile as tile
    from concourse.tile_rust import add_dep_helper

    f32 = mybir.dt.float32
    nc = bacc.Bacc()

    x_d = nc.declare_dram_parameter("xloc", [BLOC, nsteps, V], f32, isOutput=False)
    consts_d = nc.declare_dram_parameter("consts", [128, CONSTS_W], f32,
                                         isOutput=False)
    ys_d = nc.declare_dram_parameter("ys", [BLOC, nsteps, V], f32, isOutput=True)

    XCH = min(128, nsteps)          # x prefetch / A5 chunk, in steps
    nxch = (nsteps + XCH - 1) // XCH
    YCH = min(64, nsteps)           # ys flush chunk, in steps
    RING = 2 * YCH

    sub = mybir.AluOpType.subtract
    mult = mybir.AluOpType.mult
    add = mybir.AluOpType.add
    is_ge = mybir.AluOpType.is_ge
    is_eq = mybir.AluOpType.is_equal
    axX = mybir.AxisListType.X

    with tile.TileContext(nc) as tc:
        with (
            tc.tile_pool(name="persist", bufs=1) as pp,
            tc.tile_pool(name="xstage", bufs=2) as xp,
            tc.tile_pool(name="psum", bufs=1, space="PSUM") as pspool,
        ):
            # constants: one blob, one DMA, one semaphore (ISA structs have
            # few wait slots; scattered const DMAs would overflow them)
            cblob = pp.tile([128, CONSTS_W], f32, tag="cblob")
            nc.sync.dma_start(out=cblob[:], in_=consts_d[:])
            o = _COFF
            emb32 = cblob[0:32, o['emb32']:o['emb32'] + D]
            wout = cblob[:, o['wout']:o['wout'] + J2]
            brow128 = cblob[:, o['brow128']:o['brow128'] + J2]
            cmab = cblob[:, o['cmab']:o['cmab'] + J2]
            c2 = cblob[:, o['c2']:o['c2'] + V]
            iota32 = cblob[:, o['iota32']:o['iota32'] + 32]
            c20j = cblob[:, o['c20j']:o['c20j'] + V]
            ewb32 = cblob[0:32, o['ewb32']:o['ewb32'] + J2]
            ident = cblob[:, o['ident']:o['ident'] + 128]
            tc.strict_bb_all_engine_barrier()

            # persistent working tiles
            a5 = pp.tile([BLOC, nsteps], f32, tag="a5")
            ysring = pp.tile([BLOC, RING, 32], f32, tag="ysring")
            stateT = pp.tile([D, BLOC], f32, tag="stateT")
            dfm = pp.tile([BLOC, J2], f32, tag="dfm")
            msk = pp.tile([BLOC, 3 * V], f32, tag="msk")
            mx = pp.tile([BLOC, 2], f32, tag="mx")
            rr3 = pp.tile([BLOC, 3], f32, tag="rr3")
            pf = pp.tile([BLOC, 1], f32, tag="pf")
            tpf = pp.tile([BLOC, 1], f32, tag="tpf")
            yf = pp.tile([BLOC, 1], f32, tag="yf")
            qf = pp.tile([BLOC, 1], f32, tag="qf")
            q20f = pp.tile([BLOC, 1], f32, tag="q20f")
            ohT = pp.tile([32, BLOC], f32, tag="ohT")

            # full-bank tiles (512 f32 = one PSUM bank) so partition-sliced
            # matmul outputs keep the zero-region bookkeeping consistent
            net_ps = [pspool.tile([BLOC, 512], f32, tag=f"net_ps{i}",
                                  name=f"net_ps{i}")[:, 0:J2]
                      for i in range(2)]
            ohT_ps = pspool.tile([32, BLOC], f32, tag="ohT_ps")
            state_ps = pspool.tile([D, BLOC], f32, tag="state_ps")

            # ---- prologue ----
            nc.gpsimd.memset(stateT[:], 0.0)
            nc.vector.tensor_copy(out=net_ps[0][:], in_=brow128[:])  # net_0 = b

            # ---- A5 pre-pass: A5[:, t] = sum_v x[:, t, v] * (20 - v) ----
            for c in range(nxch):
                t0 = c * XCH
                ncols = min(XCH, nsteps - t0)
                xt = xp.tile([BLOC, XCH, V], f32, tag="xt")
                xm = xp.tile([BLOC, XCH, V], f32, tag="xm")
                nc.sync.dma_start(out=xt[:, :ncols, :], in_=x_d[:, t0:t0 + ncols, :])
                nc.vector.tensor_tensor(
                    out=xm[:, :ncols, :], in0=xt[:, :ncols, :],
                    in1=c20j[:].unsqueeze(1).broadcast_to((BLOC, ncols, V)),
                    op=mult)
                nc.vector.reduce_sum(out=a5[:, t0:t0 + ncols], in_=xm[:, :ncols, :],
                                     axis=axX)

            # ---- the scan ----
            for t in range(nsteps):
                nA = net_ps[t % 2]
                nB = net_ps[(t + 1) % 2]

                # --- DVE: argmax encode over both heads (3 segments) ---
                nc.vector.reduce_max(out=mx[:], in_=nA[:].rearrange(
                    "b (h v) -> b h v", h=2), axis=axX)
                nc.vector.tensor_tensor(
                    out=dfm[:].rearrange("b (h v) -> b h v", h=2),
                    in0=nA[:].rearrange("b (h v) -> b h v", h=2),
                    in1=mx[:].unsqueeze(2).broadcast_to((BLOC, 2, V)),
                    op=sub)
                nc.vector.scalar_tensor_tensor(
                    out=msk[:, 0:J2], in0=dfm[:], scalar=0.0, in1=cmab[:],
                    op0=is_ge, op1=mult)
                nc.vector.scalar_tensor_tensor(
                    out=msk[:, J2:3 * V], in0=dfm[:, V:J2], scalar=0.0,
                    in1=c2[:], op0=is_ge, op1=mult)
                nc.vector.reduce_max(out=rr3[:], in_=msk[:].rearrange(
                    "b (h v) -> b h v", h=3), axis=axX)
                # --- DVE: index math, all exact fp32 (see module docstring) ---
                nc.vector.tensor_scalar(
                    out=pf[:], in0=rr3[:, 2:3], scalar1=rr3[:, 1:2], op0=sub,
                    scalar2=64.0, op1=mult)
                nc.vector.tensor_scalar(
                    out=tpf[:], in0=a5[:, t:t + 1], scalar1=rr3[:, 0:1],
                    op0=sub, scalar2=pf[:], op1=mult)
                nc.vector.tensor_scalar(
                    out=yf[:], in0=tpf[:], scalar1=0.0500030517578125,
                    op0=mult, scalar2=18.50025177001953125, op1=add)
                nc.vector.tensor_scalar(
                    out=qf[:], in0=yf[:], scalar1=8388608.0, op0=add,
                    scalar2=8388608.0, op1=sub)
                nc.vector.tensor_scalar(
                    out=q20f[:], in0=qf[:], scalar1=20.0, scalar2=None,
                    op0=mult)
                ohs = ysring[:, t % RING, :]
                nc.vector.tensor_scalar(
                    out=ohs, in0=iota32[:], scalar1=q20f[:], op0=add,
                    scalar2=tpf[:], op1=is_eq)
                # --- PE: ohT = oh.T (the DVE stream transpose crashes the
                # device in this runtime, so transpose on the PE) ---
                nc.tensor.transpose(ohT_ps[:], ohs, ident[:])
                nc.vector.tensor_copy(out=ohT[:], in_=ohT_ps[:])

                # --- ACT: stage state_t to SBUF for the next-step matmul ---
                # (t=0: state_0 = 0 comes from the prologue memset)
                if t > 0:
                    nc.scalar.copy(out=stateT[:], in_=state_ps[:])

                # --- PE: net(t+1) = state_t @ W_out (early) ++ EWb[u_t] ---
                # mm1 zeroes the bank (start=True) and runs during the argmax
                # phase; mm4 then accumulates the EWb row. The start=True
                # region-zero makes this order-critical, so pin it.
                mm1 = nc.tensor.matmul(nB[:], stateT[:], wout[:],
                                       start=True, stop=False,
                                       skip_group_check=True)
                m4 = nc.tensor.matmul(nB[:], ohT[:], ewb32[:],
                                      start=False, stop=True,
                                      skip_group_check=True)
                add_dep_helper(m4.ins, mm1.ins, sync=False,
                               reason="net accum order")
                # --- PE: state += emb[u_t] (exact row add via one-hot) ---
                nc.tensor.matmul(state_ps[:], emb32[:], ohT[:],
                                 start=(t == 0), stop=(t == nsteps - 1),
                                 skip_group_check=True)

                # flush ys every YCH steps
                if (t + 1) % YCH == 0:
                    h0 = (t + 1 - YCH) % RING
                    nc.sync.dma_start(
                        out=ys_d[:, t + 1 - YCH:t + 1, :],
                        in_=ysring[:, h0:h0 + YCH, :V])
            if nsteps % YCH:
                tdone = (nsteps // YCH) * YCH
                h0 = tdone % RING
                nc.sync.dma_start(
                    out=ys_d[:, tdone:nsteps, :],
                    in_=ysring[:, h0:h0 + (nsteps - tdone), :V])

    nc.finalize()
    return nc


def _get_module(nsteps):
    if nsteps not in _CACHE:
        _CACHE[nsteps] = _build_module(nsteps)
    return _CACHE[nsteps]


def _make_in_maps(x, emb, W_out, b, nsteps):
    f32 = np.float32
    blob = _build_consts_blob(np.asarray(emb, f32), np.asarray(W_out, f32),
                              np.asarray(b, f32))
    in_maps = []
    for c in range(NCORES):
        xl = np.ascontiguousarray(x[c * BLOC:(c + 1) * BLOC, :nsteps, :], f32)
        in_maps.append(dict(xloc=xl, consts=blob))
    return in_maps


def kernel(x, emb, W_out, b):
    from concourse.bass_utils import run_bass_kernel_spmd

    x = np.asarray(x, np.float32)
    emb = np.asarray(emb, np.float32)
    W_out = np.asarray(W_out, np.float32)
    b = np.asarray(b, np.float32)

    nsteps = x.shape[1]
    nc = _get_module(nsteps)
    in_maps = _make_in_maps(x, emb, W_out, b, nsteps)
    res = run_bass_kernel_spmd(nc, in_maps, list(range(NCORES)))
    out = np.concatenate([res.results[c]["ys"] for c in range(NCORES)], axis=0)
    return out.astype(np.float32)
